# revision 12
# baseline (speedup 1.0000x reference)
"""DRConv (dynamic region-aware conv) Trainium2 kernel, v2.

Math (per batch b, all on device):
  x_se  = 0.25*sigmoid(routing_w @ mean_hw(x) + routing_b)           # [G*T]
  Z_t   = conv3x3(x, template_t)       for t in 0..T-1               # [O, H, W]
  U     = [x_se.T | 1] contracted with exp(Alpha) over g             # [T+1, P]
  out   = (sum_t Z_t * U_t) / U_T  + bias                            # [O, H, W]
which equals the reference because the template blend commutes through
the conv (blend weights and softmax probs both act per (g, pixel)).

Sharding: data-parallel over batch B=8, one batch element per core.

v2 changes vs v1:
  - x ships from host already scattered into the padded 57-pitch
    planes (bf16), so no on-device memset/copy staging.
  - Alpha ships zero-padded into the plane layout; exp(0)=1 supplies
    the pad-pixel softmax denominator for free (single ACT, no memset).
  - bf16 warmup matmuls (8) instead of 30 fp32 ones.
  - input DMA issues spread across Sync/Scalar/GpSimd queues.
  - no PE transpose: each tile's [px, o] accumulator is DMA'd straight
    to DRAM; the [px,o] -> [o,hw] flip happens in the host gather.
"""

import ml_dtypes
import numpy as np

import concourse.bass as bass
import concourse.mybir as mybir
from concourse import bacc
from concourse.tile import TileContext
from concourse.bass_utils import run_bass_kernel_spmd

# problem constants
C = 128          # in channels
O = 128          # out channels
H = W = 56
G = 8            # groups
T = 8            # num weight templates
WP = 57          # padded row width (one shared pad column)
HPAD = 58        # one pad row top and bottom
NPIX = HPAD * WP  # 3306
GUARD = 64       # front guard in the x buffer for negative conv shifts
PT0 = WP         # first pixel-tile starts at padded row 1
NT = 25          # 25 tiles of 128 px cover pf [57, 3257) > last valid 3247
AFREE = 3328     # alpha plane free size (NPIX rounded up)
XB1 = 29 * WP    # pf origin of band-B buffer
NB0 = GUARD + 32 * WP   # band-A buffer cols (pf -GUARD .. 1824)
NB1 = 30 * WP           # band-B buffer cols (pf 1653 .. 3363)
NCORES = 8
NWARM = 12       # bf16 warmup matmuls

_cache = {}


def _delta(ij):
    i, j = divmod(ij, 3)
    return (i - 1) * WP + (j - 1)


def _build(use_alpha: int):
    f32 = mybir.dt.float32
    bf16 = mybir.dt.bfloat16

    nc = bacc.Bacc("TRN2", target_bir_lowering=False, debug=False,
                   num_devices=NCORES)

    x0_d = nc.dram_tensor("x0", [C, NB0], bf16, kind="ExternalInput")
    x1_d = nc.dram_tensor("x1", [C, NB1], bf16, kind="ExternalInput")
    if use_alpha:
        alpha_d = nc.dram_tensor("alpha", [G, AFREE], f32,
                                 kind="ExternalInput")
    else:
        # hard routing: host ships ea = one_hot(mask) directly
        ea_d = nc.dram_tensor("ea", [G, AFREE], bf16, kind="ExternalInput")
    tmpl_d = nc.dram_tensor("tmpl", [9, C, T * O], bf16, kind="ExternalInput")
    rwt_d = nc.dram_tensor("rwt", [C, G * T], f32, kind="ExternalInput")
    rb_d = nc.dram_tensor("rb", [G * T], f32, kind="ExternalInput")
    bias_d = nc.dram_tensor("bias", [O], f32, kind="ExternalInput")
    out_d = nc.dram_tensor("out", [NT, 128, O], f32, kind="ExternalOutput")

    with TileContext(nc) as tc:
        with (
            tc.tile_pool(name="big", bufs=1) as big,
            tc.tile_pool(name="consts", bufs=1) as consts,
            tc.tile_pool(name="acc", bufs=3) as accp,
            tc.tile_pool(name="upool", bufs=5) as upool,
            tc.tile_pool(name="wzp", bufs=2) as wzp,
            tc.tile_pool(name="zps", bufs=3, space="PSUM") as zps,
            tc.tile_pool(name="ups", bufs=2, space="PSUM") as ups,
        ):
            # ---- PE warmup on a zeroed bf16 tile while inputs stream ----
            warmz = consts.tile([128, 512], bf16)
            nc.vector.memset(warmz[:], 0.0)
            warm = zps.tile([128, 512], f32, tag="zp0", name="warm")
            for _ in range(NWARM):
                nc.tensor.matmul(warm[:], lhsT=warmz[:, 0:128], rhs=warmz[:])

            # ---- input DMAs, issues spread across engine queues ----
            xbf0 = big.tile([C, NB0], bf16)
            nc.sync.dma_start(out=xbf0[:], in_=x0_d[:])
            xbf1 = big.tile([C, NB1], bf16)
            nc.sync.dma_start(out=xbf1[:], in_=x1_d[:])
            if use_alpha:
                ast = big.tile([G, AFREE], f32)
                nc.sync.dma_start(out=ast[:], in_=alpha_d[:])

            tbf = []
            for ij in range(9):
                tb = big.tile([C, T * O], bf16, name=f"tbf{ij}")
                eng = nc.scalar if ij < 4 else nc.gpsimd
                eng.dma_start(out=tb[:], in_=tmpl_d[ij])
                tbf.append(tb)

            rwt = consts.tile([C, G * T], f32)
            nc.gpsimd.dma_start(out=rwt[:], in_=rwt_d[:])
            rb = consts.tile([G * T, 1], f32)
            nc.gpsimd.dma_start(out=rb[:], in_=rb_d[:])
            bias_rep = consts.tile([128, O], f32)
            nc.scalar.dma_start(
                out=bias_rep[:],
                in_=bass.AP(tensor=bias_d, offset=0, ap=[[0, 128], [1, O]]),
            )

            # ---- routing: GAP -> fc -> sigmoid ----
            # band A sum covers image rows 0..30 (pads/guard are zero);
            # band B slice skips its first 3 rows (28..30, already in A)
            xsum = consts.tile([C, 1], f32)
            xsum0 = consts.tile([C, 1], f32)
            nc.vector.tensor_reduce(
                out=xsum0[:], in_=xbf0[:],
                axis=mybir.AxisListType.X, op=mybir.AluOpType.add)
            nc.vector.tensor_reduce(
                out=xsum[:], in_=xbf1[:, 3 * WP:],
                axis=mybir.AxisListType.X, op=mybir.AluOpType.add)
            nc.vector.tensor_add(xsum[:], xsum[:], xsum0[:])

            zr = ups.tile([G * T, 1], f32, tag="up")
            nc.tensor.matmul(zr[:], lhsT=rwt[:], rhs=xsum[:])
            xse = consts.tile([G * T, 1], f32)
            nc.scalar.activation(xse[:], zr[:],
                                 mybir.ActivationFunctionType.Sigmoid,
                                 bias=rb[:], scale=1.0 / (H * W))
            xse4 = consts.tile([G * T, 1], bf16)
            nc.vector.tensor_scalar_mul(xse4[:], xse[:], 2.0 / T)

            # lhsT_U [g, T+1]: cols 0..T-1 = x_se[g, t], col T = 1.0
            lhsu = consts.tile([G, T + 1], bf16)
            nc.vector.memset(lhsu[:, T:T + 1], 1.0)
            nc.sync.dma_start(out=lhsu[:, 0:T], in_=xse4[:])

            # ---- routing numerators: ea = exp(alpha), pads exp(0)=1 ----
            ea = big.tile([G, AFREE], bf16)
            if use_alpha:
                nc.scalar.activation(ea[:], ast[:],
                                     mybir.ActivationFunctionType.Exp)
            else:
                nc.sync.dma_start(out=ea[:], in_=ea_d[:])

            # ---- U path, software-pipelined ULEAD tiles ahead of the
            # conv stream so its DVE deps never stall the PE ----
            ULEAD = 3
            usbs = [None] * NT

            def u_tile(j):
                b = PT0 + 128 * j
                up = ups.tile([128, T + 1], f32, tag="up", name=f"up{j}")
                nc.tensor.matmul(up[:], lhsT=ea[:, b:b + 128], rhs=lhsu[:])
                rcol = upool.tile([128, 1], f32, tag="rcol")
                nc.vector.reciprocal(rcol[:], up[:, T:T + 1])
                usb = upool.tile([128, T], f32, tag="usb", name=f"usb{j}")
                nc.vector.tensor_scalar_mul(usb[:], up[:, 0:T], rcol[:])
                usbs[j] = usb

            for j in range(ULEAD):
                u_tile(j)

            # ---- main loop over pixel tiles ----
            for k in range(NT):
                base = PT0 + 128 * k

                zp = [zps.tile([128, 512], f32, tag=f"zp{h}",
                               name=f"zp{h}_{k}")
                      for h in range(2)]

                def conv_half(h):
                    for ij in range(9):
                        if k <= 12:
                            lo = GUARD + base + _delta(ij)
                            xsl = xbf0[:, lo:lo + 128]
                        else:
                            lo = base - XB1 + _delta(ij)
                            xsl = xbf1[:, lo:lo + 128]
                        nc.tensor.matmul(
                            zp[h][:],
                            lhsT=xsl,
                            rhs=tbf[ij][:, h * 512:(h + 1) * 512],
                            start=(ij == 0), stop=(ij == 8))

                usb = usbs[k]
                conv_half(0)
                conv_half(1)
                if k + ULEAD < NT:
                    u_tile(k + ULEAD)

                # mixing: ScalarE scales the h=0 half into wz (starts as
                # soon as zp[0] stops, overlapping the h=1 matmuls); DVE
                # reduces wz, runs the 4-term chain on zp[1], and combines
                wz = wzp.tile([128, 512], bf16, tag="wz")
                for t in range(4):
                    nc.scalar.activation(
                        wz[:, t * 128:(t + 1) * 128],
                        zp[0][:, t * 128:(t + 1) * 128],
                        mybir.ActivationFunctionType.Copy,
                        scale=usb[:, t:t + 1])
                accB = accp.tile([128, O], f32, tag="accB")
                nc.vector.tensor_reduce(
                    out=accB[:],
                    in_=wz[:].rearrange("p (t o) -> p o t", t=4),
                    axis=mybir.AxisListType.X, op=mybir.AluOpType.add)

                acc = accp.tile([128, O], f32, tag="acc")
                for t in range(4, T):
                    nc.vector.scalar_tensor_tensor(
                        out=acc[:],
                        in0=zp[1][:, (t - 4) * 128:(t - 3) * 128],
                        scalar=usb[:, t:t + 1],
                        in1=bias_rep[:] if t == 4 else acc[:],
                        op0=mybir.AluOpType.mult,
                        op1=mybir.AluOpType.add)
                nc.vector.tensor_add(acc[:], acc[:], accB[:])

                nc.sync.dma_start(out=out_d[k], in_=acc[:])

    nc.compile()
    return nc


def _get(use_alpha: int):
    if use_alpha not in _cache:
        _cache[use_alpha] = _build(use_alpha)
    return _cache[use_alpha]


def _in_maps(inp):
    ua = int(np.asarray(inp["use_alpha"]))
    x = np.asarray(inp["inputs"], dtype=np.float32).reshape(
        NCORES, C, H, W).astype(ml_dtypes.bfloat16)

    # band A: pf [-GUARD, 1824) = plane rows 0..31 (img rows 0..30)
    xb0 = np.zeros((NCORES, C, NB0), ml_dtypes.bfloat16)
    v0 = xb0[:, :, GUARD:].reshape(NCORES, C, 32, WP)
    v0[:, :, 1:32, 0:W] = x[:, :, 0:31, :]
    # band B: pf [1653, 3363) = plane rows 29..58 (img rows 28..55)
    xb1 = np.zeros((NCORES, C, NB1), ml_dtypes.bfloat16)
    v1 = xb1.reshape(NCORES, C, 30, WP)
    v1[:, :, 0:28, 0:W] = x[:, :, 28:56, :]

    if ua:
        # alpha scattered into the plane; zero pads -> exp=1
        al = np.zeros((NCORES, G, AFREE), np.float32)
        va = al[:, :, 0:NPIX].reshape(NCORES, G, HPAD, WP)
        va[:, :, 1:57, 0:W] = np.asarray(inp["Alpha"], dtype=np.float32)
    else:
        # hard routing: ea = one_hot(mask), pads 1.0 (any nonzero denom)
        mk = np.asarray(inp["mask"]).reshape(NCORES, H, W)
        ea = np.ones((NCORES, G, AFREE), np.float32)
        ve = ea[:, :, 0:NPIX].reshape(NCORES, G, HPAD, WP)
        ve[:, :, 1:57, 0:W] = (
            mk[:, None, :, :] == np.arange(G)[None, :, None, None])
        al = ea.astype(ml_dtypes.bfloat16)

    # [O*C*3*3, T] -> [(i,j), c, t*O + o]
    tmpl = np.asarray(inp["weight_templates"], dtype=np.float32).reshape(
        O, C, 3, 3, T).transpose(2, 3, 1, 4, 0).reshape(9, C, T * O)
    tmpl = np.ascontiguousarray(tmpl).astype(ml_dtypes.bfloat16)
    rwt = np.ascontiguousarray(
        np.asarray(inp["routing_w"], dtype=np.float32).T)
    rb = np.ascontiguousarray(np.asarray(inp["routing_b"], dtype=np.float32))
    bias = np.ascontiguousarray(np.asarray(inp["bias"], dtype=np.float32))

    akey = "alpha" if ua else "ea"
    return [
        {"x0": np.ascontiguousarray(xb0[b]),
         "x1": np.ascontiguousarray(xb1[b]),
         akey: np.ascontiguousarray(al[b]),
         "tmpl": tmpl, "rwt": rwt, "rb": rb, "bias": bias}
        for b in range(NCORES)
    ]


def kernel(inputs, mask, Alpha, weight_templates, routing_w, routing_b, bias,
           use_alpha):
    ua = int(np.asarray(use_alpha))
    nc = _get(ua)
    in_maps = _in_maps(dict(inputs=inputs, mask=mask, Alpha=Alpha,
                            weight_templates=weight_templates,
                            routing_w=routing_w, routing_b=routing_b,
                            bias=bias, use_alpha=use_alpha))
    res = run_bass_kernel_spmd(nc, in_maps, list(range(NCORES)))
    out = np.empty((NCORES, O, H, W), np.float32)
    plane = np.zeros((O, NPIX), np.float32)
    for b in range(NCORES):
        tiles = res.results[b]["out"].reshape(NT * 128, O)  # [pf-PT0, o]
        plane[:, PT0:PT0 + NT * 128] = tiles.T
        out[b] = plane.reshape(O, HPAD, WP)[:, 1:57, 0:W]
    return np.ascontiguousarray(out)
